# revision 1
# baseline (speedup 1.0000x reference)
"""Trainium2 Bass kernel for nn_Interpolator (quadratic-form kernel interpolation).

Math (T=8192 targets, C=8192 contexts, D=64, DY=32):
    S = W + W^T
    scores[t,c] = (z_t - z_c)^T W (z_t - z_c)
                = q_tt[t] + q_cc[c] - z_t^T S z_c
    theta = exp(-scores);  out = (theta @ y_context) / theta.sum(-1, keepdim)

The q_tt[t] term is a per-target constant factor exp(-q_tt[t]) on the whole
theta row, which cancels exactly in the normalization -> dropped. Only
q_cc[c] (a per-context weight) is computed.

Sharding: data-parallel over targets; each of the 8 cores takes T/8 = 1024
targets and the full context set.

Per-core device program (in the "transposed" domain, D on partitions):
  - big matmul (K=65, fp16 operands, fp32 PSUM accumulate):
      psum[c,t] = sum_d zcT[d,c]*zsT[d,t] + (-q_cc[c])*1  = cross - q_cc
    where zsT = S^T ztT. fp16, not fp32 (fp32 matmuls run as 2 LOW/HIGH
    passes at half stream rate = ~4x cost) and not bf16 (8-bit mantissa
    costs ~3e-2 absmax on the output; fp16 keeps it ~5e-3 of scale).
  - theta^T = Exp(psum) on the scalar engine -> bf16 (theta spans ~e^59,
    needs bf16 range). ACT is the roofline: 8.4M exps/core @ 1/lane/cycle.
  - second matmul (bf16) accumulates out2T[j,t] += y_aug[c,j]*theta^T[c,t]
    over 64 context chunks; y_aug col 32 = ones gives the denominator.
  - A ~5us dependency-free matmul burst at kernel start flips the PE HAM
    clock-gate to 8/8 (the main loop alone never bootstraps it), and a
    dummy Exp preloads the ACT spline table during the DMA phase.
  - The q_cc prelude (zs = S^T zc, mul, ones-reduce, row copy) is cut into
    16 half-block chains on 2 dedicated PSUM banks and emitted interleaved
    with the first 15 main-loop chunks so it rides in PE/DVE slack instead
    of serializing ahead of the loop.
Host: shard/transpose/cast inputs (layout only), concat per-core [33,1024]
outputs, divide numerator rows by the denominator row.
"""

import ml_dtypes
import numpy as np

import concourse.bacc as bacc
import concourse.bass as bass
import concourse.mybir as mybir
import concourse.tile as tile
from concourse.bass_utils import run_bass_kernel_spmd

F32 = mybir.dt.float32
F16 = mybir.dt.float16
BF16 = mybir.dt.bfloat16

T, C, D, DY = 8192, 8192, 64, 32
NCORES = 8
TL = T // NCORES          # 1024 targets per core
KAUG = D + 1              # 65 contraction rows: 64 z-dims + (-q_cc | ones)
NCHUNK = C // 128         # 64 context chunks of 128
NBLK = C // 1024          # 8 column blocks of the lhsT slab
HALF = 512                # PSUM-bank-sized matmul free dim
NWARM = 16


def _build_kernel_body(tc: tile.TileContext):
    nc = tc.nc
    Exp = mybir.ActivationFunctionType.Exp

    zt_d = nc.dram_tensor("ztt", [D, TL], F16, kind="ExternalInput")
    y_d = [
        nc.dram_tensor(f"yck{i}", [128, NCHUNK * DY // 2], BF16, kind="ExternalInput")
        for i in range(2)
    ]
    w_d = nc.dram_tensor("w", [D, D], F32, kind="ExternalInput")
    wt_d = nc.dram_tensor("wt", [D, D], F32, kind="ExternalInput")
    zc_d = [
        nc.dram_tensor(f"zct{b}", [D, 1024], F16, kind="ExternalInput")
        for b in range(NBLK)
    ]
    out_d = nc.dram_tensor("out", [DY + 1, TL], F32, kind="ExternalOutput")

    with (
        tc.tile_pool(name="singles", bufs=1) as singles,
        tc.tile_pool(name="spre", bufs=2) as spre,
        tc.tile_pool(name="theta", bufs=3) as thp,
        tc.tile_pool(name="psum", bufs=1, space="PSUM") as pps,
        tc.tile_pool(name="out2", bufs=1, space="PSUM") as o2p,
    ):
        # ---- resident SBUF slabs ----
        # LC[b]: [65, 1024] fp16 lhsT block: rows 0..63 = zcT (host-permuted:
        # chunk j partition p <-> original context p*64+j), row 64 = -q_cc.
        LC = [singles.tile([KAUG, 1024], F16, name=f"lc{b}") for b in range(NBLK)]
        RT = singles.tile([KAUG, TL], F16, name="rt")
        ZT = singles.tile([D, TL], F16, name="zt")
        YT = singles.tile([128, NCHUNK * DY], BF16, name="yt")
        YA = singles.tile([128, NCHUNK, DY + 1], BF16, name="ya")
        WS = singles.tile([D, D], F32, name="ws")
        WTT = singles.tile([D, D], F32, name="wtt")
        SSF = singles.tile([D, D], F32, name="ssf")
        SS = singles.tile([D, D], F16, name="ss")
        NH = singles.tile([D, 1], F16, name="nh")
        OSB = singles.tile([DY + 1, TL], F32, name="osb")
        WRM = singles.tile([128, HALF], BF16, name="wrm")
        EXD = singles.tile([D, 1], F32, name="exd")

        # ---- PE warm-up burst + ACT exp-table preload ----
        # Filler matmuls (dependency-free, own PSUM bank) bridge every
        # PE-idle window in the head so HAM warms early and never sees a
        # ~3.4us MID window before the main loop's dense stream takes over.
        wps = pps.tile([128, HALF], F32, tag="warm")

        def fill(n):
            for _ in range(n):
                nc.tensor.matmul(wps, WRM[:, 0:128], WRM, start=True, stop=True)

        nc.vector.memset(WRM, 0.5)
        fill(NWARM)
        nc.vector.memset(EXD, 0.0)
        nc.scalar.activation(EXD, EXD, Exp)

        # ---- loads (order: prelude-critical tensors first) ----
        nc.sync.dma_start(out=WS, in_=w_d.ap())
        nc.sync.dma_start(out=WTT, in_=wt_d.ap())
        nc.sync.dma_start(out=ZT, in_=zt_d.ap())
        nc.sync.dma_start(out=LC[0][:D, :], in_=zc_d[0].ap())
        half_y = NCHUNK * DY // 2
        nc.sync.dma_start(out=YT[:, :half_y], in_=y_d[0].ap())
        nc.sync.dma_start(out=YT[:, half_y:], in_=y_d[1].ap())
        for b in range(1, NBLK):
            nc.sync.dma_start(out=LC[b][:D, :], in_=zc_d[b].ap())

        # DVE emission order matters: the queue is strict FIFO, so emit in
        # expected-readiness order (an op waiting on a slow DMA would
        # head-of-line-block everything behind it).
        nc.vector.tensor_add(SSF, WS, WTT)  # S = W + W^T
        nc.vector.tensor_copy(SS, SSF)      # -> fp16
        nc.vector.memset(NH, -0.5)
        nc.vector.memset(RT[D : D + 1, :], 1.0)  # ones row @64

        # ---- prelude: RT = [zsT; ones] ----
        zs_ps = pps.tile([128, TL], F32, tag="sc", bufs=2)
        for h in range(TL // HALF):
            sl = slice(h * HALF, (h + 1) * HALF)
            nc.tensor.matmul(zs_ps[:D, sl], SS, ZT[:, sl], start=True, stop=True)
        nc.vector.tensor_copy(RT[:D, :], zs_ps[:D, :])  # -> fp16

        # ---- q_cc half-block chain ----
        # One [128, HALF] PSUM tile per half: zs lands in partitions 0..63,
        # the ones-reduce (-q_cc) in partition 64 of the same bank; pool
        # slot reuse (bufs=1) serializes bank hand-off safely.
        def qcc_half(h, nfill=0):
            b, hh = divmod(h, 2)
            sl = slice(hh * HALF, (hh + 1) * HALF)
            ps = pps.tile([128, HALF], F32, tag="pre")
            nc.tensor.matmul(ps[:D, :], SS, LC[b][:D, sl], start=True, stop=True)
            if nfill:
                fill(nfill)
            mc = spre.tile([D, HALF], F16, tag="m")
            nc.vector.tensor_mul(mc, ps[:D, :], LC[b][:D, sl])
            nc.tensor.matmul(ps[D : D + 1, :], NH, mc, start=True, stop=True)
            nc.vector.tensor_copy(LC[b][D : D + 1, sl], ps[D : D + 1, :])

        qcc_half(0, nfill=4)
        qcc_half(1, nfill=4)
        # Bridge the remaining prelude DVE chain (RT cast + q_cc mul/copy)
        # so the PE issue stream stays gapless until the main loop takes
        # over — HAM re-throttles on any idle in its ~3.4us window.
        fill(18)

        # y_aug: [128, chunk, 33]; col 32 = 1.0 (denominator trick).
        # Emitted late: waits on the big y DMA, must not block the q_cc ops.
        nc.vector.tensor_copy(
            YA[:, :, 0:DY], YT[:, :].rearrange("p (j d) -> p j d", d=DY)
        )
        nc.vector.memset(YA[:, :, DY : DY + 1], 1.0)

        # ---- main loop over 64 context chunks ----
        o2 = o2p.tile([DY + 1, TL], F32)
        for j in range(NCHUNK):
            b, p0 = divmod(j * 128, 1024)
            lhsT = LC[b][:, p0 : p0 + 128]
            sc = pps.tile([128, TL], F32, tag="sc", bufs=2)
            for h in range(TL // HALF):
                sl = slice(h * HALF, (h + 1) * HALF)
                nc.tensor.matmul(sc[:, sl], lhsT, RT[:, sl], start=True, stop=True)
            th = thp.tile([128, TL], BF16)
            nc.scalar.activation(th, sc, Exp)
            for h in range(TL // HALF):
                sl = slice(h * HALF, (h + 1) * HALF)
                nc.tensor.matmul(
                    o2[:, sl],
                    YA[:, j, :],
                    th[:, sl],
                    start=(j == 0),
                    stop=(j == NCHUNK - 1),
                )
            if 1 <= j <= 14:
                qcc_half(j + 1)

        # ---- epilogue ----
        nc.vector.tensor_copy(OSB, o2)
        nc.sync.dma_start(out=out_d.ap(), in_=OSB)


_CACHED = None


def _get_nc():
    global _CACHED
    if _CACHED is None:
        nc = bacc.Bacc(
            "TRN2",
            target_bir_lowering=False,
            debug=False,
            enable_asserts=False,
        )
        with tile.TileContext(nc) as tc:
            _build_kernel_body(tc)
        nc.compile()
        _CACHED = nc
    return _CACHED


def make_in_maps(z_context, y_context, z_target, W):
    """Host-side layout prep (transpose/reshape/cast only) + sharding."""
    z_context = np.asarray(z_context, dtype=np.float32)
    y_context = np.asarray(y_context, dtype=np.float32)
    z_target = np.asarray(z_target, dtype=np.float32)
    W = np.asarray(W, dtype=np.float32)

    # Permute contexts so chunk j partition p holds original context p*64+j;
    # keeps both the zcT slab and the y slab DMA-contiguous.
    zcT = z_context.T.astype(np.float16)  # [64, 8192]
    # position q = j*128 + p  <-  context p*64 + j
    zc_perm = np.ascontiguousarray(
        zcT.reshape(D, 128, NCHUNK).transpose(0, 2, 1).reshape(D, C)
    )
    zc_blocks = [
        np.ascontiguousarray(zc_perm[:, b * 1024 : (b + 1) * 1024])
        for b in range(NBLK)
    ]
    # y in the same permuted order: row p of the SBUF tile holds contexts
    # p*64 + j for j in 0..63 -> plain reshape of the original y.
    yck = y_context.reshape(128, NCHUNK * DY).astype(ml_dtypes.bfloat16)
    half_y = NCHUNK * DY // 2
    yck0 = np.ascontiguousarray(yck[:, :half_y])
    yck1 = np.ascontiguousarray(yck[:, half_y:])
    wt = np.ascontiguousarray(W.T)

    in_maps = []
    for i in range(NCORES):
        ztT = np.ascontiguousarray(
            z_target[i * TL : (i + 1) * TL].T.astype(np.float16)
        )
        m = {"ztt": ztT, "yck0": yck0, "yck1": yck1, "w": W, "wt": wt}
        for b in range(NBLK):
            m[f"zct{b}"] = zc_blocks[b]
        in_maps.append(m)
    return in_maps


def postprocess(results):
    """Gather per-core [33, TL] outputs -> full (T, DY) normalized output."""
    allT = np.concatenate([r["out"].T for r in results], axis=0)  # [T, 33]
    return (allT[:, :DY] / allT[:, DY : DY + 1]).astype(np.float32)


def run(in_maps, **kwargs):
    nc = _get_nc()
    return run_bass_kernel_spmd(nc, in_maps, core_ids=list(range(NCORES)), **kwargs)


def kernel(z_context, y_context, z_target, W):
    in_maps = make_in_maps(z_context, y_context, z_target, W)
    res = run(in_maps)
    return postprocess(res.results)



# revision 7
# speedup vs baseline: 1.0120x; 1.0120x over previous
"""Trainium2 Bass kernel for nn_Interpolator (quadratic-form kernel interpolation).

Math (T=8192 targets, C=8192 contexts, D=64, DY=32):
    S = W + W^T
    scores[t,c] = (z_t - z_c)^T W (z_t - z_c)
                = q_tt[t] + q_cc[c] - z_t^T S z_c
    theta = exp(-scores);  out = (theta @ y_context) / theta.sum(-1, keepdim)

q_tt[t] is a per-target factor on the whole theta row -> cancels in the
normalization -> dropped. q_cc[c] is a per-context factor: instead of a 65th
matmul contraction row (as in the v1 kernel), it is folded into the y-reduce
weights on device:  y'[c,:] = y_aug[c,:] * exp(-q_cc[c]), so
    theta' = exp(z_t^T S z_c)           (plain exp of the cross matmul)
    out2   = y'^T @ theta'              (identical product, fp-rounding aside)
This makes every matmul K=64 and lets one ACTIVATE span whole chunks with no
per-partition bias.

Sharding: data-parallel over targets; each of 8 cores takes T/8 = 1024 targets
(2 passes x 512) and the full context set (64 chunks of 128).

Per-core engine plan (ACT is the roofline: 8.4M exps @ 1 lane-elem/cycle
@1.2GHz = 54.6us + ~260ns/instruction overhead):
  - cross matmuls are K=64 -> 64x128 PE row-tiles: even chunks use partitions
    0-63 (tile T0), odd chunks partitions 64-127 (T8). Consecutive chunks
    stream concurrently and each LDWEIGHTS loads on the idle tile, keeping
    weight swaps off the PE critical path.
  - exp: one ACTIVATE per group of 2 chunk-halves (N=1024 from PSUM).
  - y-reduce: K=128 full-array matmuls into a single [33,512] PSUM bank per
    pass; row 32 of y' is exp(-q_cc) itself (= ones * e^{-q_cc}) giving the
    denominator.
  - q_cc on device: per chunk, zsn = LC_chunk^T @ S ([128c,64d] PSUM) then a
    DVE tensor_tensor_reduce against a natural-layout z_context slab with
    scale -0.5 accumulates -q_cc[chunk] straight into Q[:,j] ([128,64] SBUF,
    per-partition layout). One [128,64] ACT exp -> EQ, one DVE broadcast
    multiply scales YA. These ride in PE/DVE slack inside the main loop; the
    y-matmul stream is emission-delayed ~16 groups so nothing stalls on EQ.
  - head: input DMAs fan out across 5 engine queues; a dependency-free matmul
    burst flips the PE HAM clock gate to 8/8 and a dummy Exp preloads the ACT
    spline table during the DMA phase.
Host: shard/transpose/cast/duplicate inputs (layout only), concat per-core
[33,1024] outputs, divide numerator rows by the denominator row.
"""

import ml_dtypes
import numpy as np

import concourse.bacc as bacc
import concourse.bass as bass
import concourse.mybir as mybir
import concourse.tile as tile
from concourse.bass_utils import run_bass_kernel_spmd

F32 = mybir.dt.float32
F16 = mybir.dt.float16
BF16 = mybir.dt.bfloat16

T, C, D, DY = 8192, 8192, 64, 32
NCORES = 8
TL = T // NCORES          # 1024 targets per core
TH = TL // 2              # 512 targets per pass
NCHUNK = C // 128         # 64 context chunks of 128
NGRP = 32                 # groups of 2 chunks per pass
NWARM = 16
EQ_GROUP = 14             # global group after which EQ exp + YA scale emit
QCC_PER_GROUP = 6         # q_cc chunks interleaved per early group
NTH = 18                  # theta ring depth (covers the y emission lag)


def _build_kernel_body(tc: tile.TileContext):
    nc = tc.nc
    Exp = mybir.ActivationFunctionType.Exp
    Mul = mybir.AluOpType.mult
    Add = mybir.AluOpType.add

    X = mybir.AxisListType.X
    lce_d = nc.dram_tensor("lce", [D, NCHUNK // 2, 128], F16, kind="ExternalInput")
    lco_d = nc.dram_tensor("lco", [D, NCHUNK // 2, 128], F16, kind="ExternalInput")
    zcn_d = nc.dram_tensor("zcn", [128, NCHUNK, D], F16, kind="ExternalInput")
    ztd_d = nc.dram_tensor("ztd", [128, TL], F16, kind="ExternalInput")
    wd_d = nc.dram_tensor("wd", [128, D], F32, kind="ExternalInput")
    wtd_d = nc.dram_tensor("wtd", [128, D], F32, kind="ExternalInput")
    yad_d = nc.dram_tensor("yad", [128, DY, NCHUNK], BF16, kind="ExternalInput")
    out_d = nc.dram_tensor("out", [DY + 1, TL], F32, kind="ExternalOutput")

    with (
        tc.tile_pool(name="singles", bufs=1) as singles,
        tc.tile_pool(name="theta", bufs=NTH) as thp,
        tc.tile_pool(name="o2", bufs=1, space="PSUM") as o2p,
    ):
        # ---- resident SBUF slabs ----
        LCF = singles.tile([128, NCHUNK // 2, 128], F16, name="lcf")
        LCN = singles.tile([128, NCHUNK, D], F16, name="lcn")
        ZT = singles.tile([128, TL], F16, name="zt")
        RT = singles.tile([128, TL], F16, name="rt")
        WD = singles.tile([128, D], F32, name="wd")
        WTD = singles.tile([128, D], F32, name="wtd")
        SSF = singles.tile([128, D], F32, name="ssf")
        SS = singles.tile([128, D], F16, name="ss")
        SSQ = singles.tile([128, D], F16, name="ssq")
        P2 = singles.tile([128, QCC_PER_GROUP, D], F32, name="p2")
        YA = singles.tile([128, DY + 1, NCHUNK], BF16, name="ya")
        Q = singles.tile([128, NCHUNK], F32, name="q")
        EQ = singles.tile([128, NCHUNK], F32, name="eq")
        OSB = singles.tile([DY + 1, TL], F32, name="osb")
        WRM = singles.tile([128, 512], BF16, name="wrm")
        EXD = singles.tile([128, 1], F32, name="exd")

        # ---- PE warm-up burst + ACT exp-table preload + DMA fan-out ----
        nc.vector.memset(WRM, 0.5)
        nc.vector.memset(EXD, 0.0)
        nc.scalar.activation(EXD, EXD, Exp)

        with tc.tile_pool(name="warm", bufs=1, space="PSUM") as warmp:
            wps = warmp.tile([128, 512], F32, tag="warm")

            def fill(n):
                for _ in range(n):
                    nc.tensor.matmul(wps, WRM[:, 0:128], WRM, start=True, stop=True)

            fill(NWARM)

            # loads spread across the 3 DMA-capable engine queues
            nc.sync.dma_start(out=WD, in_=wd_d.ap())
            nc.sync.dma_start(out=WTD, in_=wtd_d.ap())
            nc.scalar.dma_start(out=ZT, in_=ztd_d.ap())
            nc.scalar.dma_start(out=LCF[0:D, :, :], in_=lce_d.ap())
            nc.sync.dma_start(out=LCF[D:128, :, :], in_=lco_d.ap())
            nc.gpsimd.dma_start(out=LCN, in_=zcn_d.ap())
            nc.sync.dma_start(out=YA[:, 0:DY, :], in_=yad_d.ap())

            nc.vector.tensor_add(SSF, WD, WTD)   # S = W + W^T (both halves)
            nc.vector.tensor_copy(SS, SSF)       # -> fp16
            nc.vector.tensor_scalar_mul(SSQ, SSF, -0.5)  # -S/2 (q_cc matmul)
            nc.vector.memset(YA[:, DY : DY + 1, :], 1.0)  # denominator row

            fill(4)

            # ---- prelude: RT = zsT = S^T ztT, duplicated on both halves ----
            with tc.tile_pool(name="prel", bufs=2, space="PSUM") as prelp:
                zpA = prelp.tile([128, TL], F32, tag="zp")
                zpB = prelp.tile([128, TL], F32, tag="zp")
                for h in range(2):
                    sl = slice(h * TH, (h + 1) * TH)
                    nc.tensor.matmul(
                        zpA[0:D, sl], SS[0:D, :], ZT[0:D, sl], start=True, stop=True
                    )
                fill(2)
                for h in range(2):
                    sl = slice(h * TH, (h + 1) * TH)
                    nc.tensor.matmul(
                        zpB[D:128, sl], SS[D:128, :], ZT[D:128, sl],
                        start=True, stop=True,
                    )
                fill(6)
                nc.vector.tensor_copy(RT[0:D, :], zpA[0:D, :])
                nc.vector.tensor_copy(RT[D:128, :], zpB[D:128, :])
                fill(8)

        # ---- main loop: 2 passes x 32 groups of 2 chunks ----
        with (
            tc.tile_pool(name="sc", bufs=2, space="PSUM") as scp,
            tc.tile_pool(name="zsn", bufs=2, space="PSUM") as znp,
        ):
            ths = [None] * (2 * NGRP)
            o2 = [None, None]
            jq = 0        # next q_cc chunk
            ydone = 0     # next y group (global)

            def emit_y_group(ygg):
                p, gl = divmod(ygg, NGRP)
                for k in range(2):
                    s = 2 * gl + k
                    nc.tensor.matmul(
                        o2[p],
                        YA[:, :, s],
                        ths[ygg][:, k * TH : (k + 1) * TH],
                        start=(s == 0),
                        stop=(s == NCHUNK - 1),
                    )

            for gg in range(2 * NGRP):
                p, g = divmod(gg, NGRP)
                sc = scp.tile([128, 2 * TH], F32, tag="sc")
                for k in range(2):
                    s = 2 * g + k
                    h = s & 1
                    hp = slice(h * D, h * D + D)
                    nc.tensor.matmul(
                        sc[:, k * TH : (k + 1) * TH],
                        LCF[hp, s >> 1, :],
                        RT[hp, p * TH : (p + 1) * TH],
                        start=True, stop=True,
                    )
                th = thp.tile([128, 2 * TH], BF16)
                nc.scalar.activation(th, sc, Exp)
                ths[gg] = th

                # q_cc chunks ride in PE/DVE slack (pass 0 only):
                # zsn = -0.5 zc S (PE), P2 = zsn .* zc (DVE), then one
                # axis-X reduce per batch -> Q[:, batch] = -q_cc.
                jq0 = jq
                for _ in range(QCC_PER_GROUP):
                    if jq >= NCHUNK:
                        break
                    h = jq & 1
                    hp = slice(h * D, h * D + D)
                    zsn = znp.tile([128, 512], F32, tag="zsn")
                    nc.tensor.matmul(
                        zsn[:, 0:D], LCF[hp, jq >> 1, :], SSQ[hp, :],
                        start=True, stop=True,
                    )
                    nc.vector.tensor_mul(
                        P2[:, jq - jq0, :], zsn[:, 0:D], LCN[:, jq, :]
                    )
                    jq += 1
                if jq > jq0:
                    nc.vector.tensor_reduce(
                        Q[:, jq0:jq], P2[:, 0 : jq - jq0, :], axis=X, op=Add
                    )

                if gg == EQ_GROUP:
                    nc.scalar.activation(EQ, Q, Exp)   # e^{-q_cc}
                    nc.vector.tensor_mul(
                        YA, YA, EQ[:, None, :].broadcast_to([128, DY + 1, NCHUNK])
                    )

                # y-reduce, emission-delayed until EQ is in; catch up 2/step
                if gg > EQ_GROUP:
                    budget = 2
                    while budget and ydone < gg - 1:
                        yp = ydone // NGRP
                        if o2[yp] is None:
                            o2[yp] = o2p.tile([DY + 1, TH], F32, tag="o2", name=f"o2_{yp}")
                        emit_y_group(ydone)
                        if ydone == NGRP - 1:
                            nc.vector.tensor_copy(OSB[:, 0:TH], o2[0])
                        ydone += 1
                        budget -= 1

            while ydone < 2 * NGRP:
                yp = ydone // NGRP
                if o2[yp] is None:
                    o2[yp] = o2p.tile([DY + 1, TH], F32, tag="o2", name=f"o2_{yp}")
                emit_y_group(ydone)
                if ydone == NGRP - 1:
                    nc.vector.tensor_copy(OSB[:, 0:TH], o2[0])
                ydone += 1
            nc.vector.tensor_copy(OSB[:, TH:TL], o2[1])
            nc.sync.dma_start(out=out_d.ap(), in_=OSB)


_CACHED = None


def _get_nc():
    global _CACHED
    if _CACHED is None:
        nc = bacc.Bacc(
            "TRN2",
            target_bir_lowering=False,
            debug=False,
            enable_asserts=False,
        )
        with tile.TileContext(nc) as tc:
            _build_kernel_body(tc)
        nc.compile()
        _CACHED = nc
    return _CACHED


def make_in_maps(z_context, y_context, z_target, W):
    """Host-side layout prep (transpose/reshape/cast/duplicate only) + shard."""
    z_context = np.asarray(z_context, dtype=np.float32)
    y_context = np.asarray(y_context, dtype=np.float32)
    z_target = np.asarray(z_target, dtype=np.float32)
    W = np.asarray(W, dtype=np.float32)

    zcT = z_context.T.astype(np.float16)               # [64, 8192]
    zc3 = zcT.reshape(D, NCHUNK, 128)
    lce = np.ascontiguousarray(zc3[:, 0::2, :])        # [64, 32, 128]
    lco = np.ascontiguousarray(zc3[:, 1::2, :])
    zcn = np.ascontiguousarray(
        z_context.reshape(NCHUNK, 128, D).transpose(1, 0, 2)
    ).astype(np.float16)                               # [128, 64, 64]
    yad = np.ascontiguousarray(
        y_context.reshape(NCHUNK, 128, DY).transpose(1, 2, 0)
    ).astype(ml_dtypes.bfloat16)                       # [128, 32, 64]
    wd = np.ascontiguousarray(np.concatenate([W, W], axis=0))       # [128, 64]
    wtd = np.ascontiguousarray(np.concatenate([W.T, W.T], axis=0))  # [128, 64]

    in_maps = []
    for i in range(NCORES):
        ztT = z_target[i * TL : (i + 1) * TL].T.astype(np.float16)  # [64, 1024]
        ztd = np.ascontiguousarray(np.concatenate([ztT, ztT], axis=0))
        in_maps.append(
            {"lce": lce, "lco": lco, "zcn": zcn, "ztd": ztd,
             "wd": wd, "wtd": wtd, "yad": yad}
        )
    return in_maps


def postprocess(results):
    """Gather per-core [33, TL] outputs -> full (T, DY) normalized output."""
    allT = np.concatenate([r["out"].T for r in results], axis=0)  # [T, 33]
    return (allT[:, :DY] / allT[:, DY : DY + 1]).astype(np.float32)


def run(in_maps, **kwargs):
    nc = _get_nc()
    return run_bass_kernel_spmd(nc, in_maps, core_ids=list(range(NCORES)), **kwargs)


def kernel(z_context, y_context, z_target, W):
    in_maps = make_in_maps(z_context, y_context, z_target, W)
    res = run(in_maps)
    return postprocess(res.results)
